# revision 7
# baseline (speedup 1.0000x reference)
"""Trainium2 Bass kernel for MiniEq2Net (gnn_message_passing).

Math (validated against the jax reference in float64, rel err ~3e-7):

Per batch b (X = x[b], [n=256, d=16]) the first eq-layer's input channels are
diag(X[:,d]) and X[:,d] outer X[:,d], so layer 1 collapses to
    G1[s] = S(s) + c'_{s,i} (row-broadcast) + delta_ij a_{s,i}
with S(s) = X diag(wt_s) X^T (symmetric, one K=64 matmul per 4-row group in a
packed (a=i%4, s) x (j) layout), and the diagonal handled exactly via tiny
[32,256] side computations (dn/dg/Hdc).  Layer 2 + pooling becomes two K=128
block-diagonal channel-mix matmuls over relu'd H and H^T plus a fused
relu-accumulate, with the diagonal / rowsum / total-sum basis terms folded
into per-partition biases and a closed-form correction.

Sharding: pure data parallel, one batch element per NeuronCore (B=8, 8 cores).
All heavy intermediates (H, H^T: 16MB) stay in SBUF; HBM traffic is ~0.5MB of
host-precomputed small operands per core.
"""

import numpy as np

N = 256          # n (graph nodes)
D = 16           # input channel count
NH = 32          # hidden channels
A = 4            # row-packing factor: partition p = a*32+s, row i = 4*g+a
G = N // A       # 64 row-groups
B = 8            # batch == cores
F32 = np.float32

_PROG_CACHE = {}


# ---------------------------------------------------------------- host side

def _percore_inputs(xb, W1, b1, W2, b2, D1, db1, D2, db2, D3, db3):
    """All small per-core operands, precomputed in float64 for accuracy."""
    X = xb.astype(np.float64)                      # [256, 16]
    n = float(N)
    sigma = X.sum(0)                               # [16]
    wt = W1[D:, :, 0] + W1[D:, :, 1]               # [16,32]
    alpha = W1[:D, :, 0] + W1[:D, :, 1] + W1[:D, :, 2]
    beta = W1[D:, :, 2]
    abias = alpha.T @ X.T + beta.T @ (X.T ** 2)    # [32,256]
    gamma = W1[:D, :, 3] / n + W1[D:, :, 3] * sigma[:, None] / n
    k = (W1[:D, :, 4].T @ (sigma / n**2)
         + W1[D:, :, 4].T @ (sigma**2 / n**2) + b1)
    cp = gamma.T @ X.T + k[:, None]                # [32,256]

    XT = X.T                                       # [16,256]
    XT4 = np.tile(XT, (A, 1))                      # [64,256]

    WtBD = np.zeros((A * D, 128))
    for a in range(A):
        WtBD[a * D:(a + 1) * D, a * NH:(a + 1) * NH] = wt
    # Xr[(a*16+d), g] = X[4g+a, d]
    Xr = X.reshape(G, A, D).transpose(1, 2, 0).reshape(A * D, G)
    I32r4 = np.tile(np.eye(NH), (1, A))            # [32,128]
    # Cpp[(a*32+s), g] = cp[s, 4g+a]
    Cpp = cp.reshape(NH, G, A).transpose(2, 0, 1).reshape(128, G)

    def blockdiag(M):
        out = np.zeros((128, 128))
        for a in range(A):
            out[a * NH:(a + 1) * NH, a * NH:(a + 1) * NH] = M
        return out

    WB0 = blockdiag(W2[:, :, 0])
    WB1 = blockdiag(W2[:, :, 1])
    WB3 = blockdiag(W2[:, :, 3] / n)
    P32 = np.tile(np.eye(NH), (A, 1))              # [128,32]

    t = {
        'XT4': XT4, 'cpm': cp, 'WtBD': WtBD, 'Xr': Xr, 'I32r4': I32r4, 'Cpp': Cpp,
        'WB0': WB0, 'WB1': WB1, 'WB3': WB3, 'P32': P32,
        'W01': W2[:, :, 0] + W2[:, :, 1], 'W22': W2[:, :, 2],
        'W24': W2[:, :, 4] / n**2,
        'wt16': wt, 'X2T': XT ** 2, 'abias': abias,
        'b2c': b2[:, None],
        'D1m': D1, 'db1m': db1[:, None],
        'D2m': D2, 'db2m': db2[:, None],
        'D3m': D3, 'db3m': db3[:, None],
    }
    return {k: np.ascontiguousarray(v, dtype=F32) for k, v in t.items()}


_SHAPES = {
    'XT4': [64, 256], 'cpm': [32, 256], 'WtBD': [64, 128], 'Xr': [64, 64], 'I32r4': [32, 128],
    'Cpp': [128, 64], 'WB0': [128, 128], 'WB1': [128, 128], 'WB3': [128, 128],
    'P32': [128, 32], 'W01': [32, 32], 'W22': [32, 32], 'W24': [32, 32],
    'wt16': [16, 32], 'X2T': [16, 256], 'abias': [32, 256], 'b2c': [32, 1],
    'D1m': [32, 128], 'db1m': [128, 1], 'D2m': [128, 128], 'db2m': [128, 1],
    'D3m': [128, 1], 'db3m': [1, 1],
}


# -------------------------------------------------------------- device side

def build_program():
    if 'nc' in _PROG_CACHE:
        return _PROG_CACHE['nc']

    import concourse.bacc as bacc
    import concourse.tile as tile
    from concourse import mybir

    f32 = mybir.dt.float32
    f32r = mybir.dt.float32r
    AF = mybir.ActivationFunctionType
    ALU = mybir.AluOpType

    nc = bacc.Bacc(trn_type="TRN2", target_bir_lowering=False)
    dram = {name: nc.dram_tensor(name, shape, f32, kind="ExternalInput")
            for name, shape in _SHAPES.items()}
    yout_d = nc.dram_tensor("yout", [1, 1], f32, kind="ExternalOutput")

    with tile.TileContext(nc) as tc:
        from contextlib import ExitStack
        ctx = ExitStack()
        consts = ctx.enter_context(tc.tile_pool(name="consts", bufs=1))
        sb = {}
        for name, shape in _SHAPES.items():
            t = consts.tile(shape, f32, name=f"sb_{name}")
            nc.default_dma_engine.dma_start(out=t, in_=dram[name].ap())
            sb[name] = t

        big = ctx.enter_context(tc.tile_pool(name="big", bufs=1))
        zero256 = big.tile([128, 256], f32, name="zero256")
        nc.vector.memset(zero256, 0.0)
        H4 = big.tile([128, G * N], f32r, name="H4")
        HT4 = big.tile([128, G * N], f32r, name="HT4")
        r4 = big.tile([128, G], f32, name="r4")
        acc = big.tile([128, G], f32, name="acc")

        lhsT_pool = ctx.enter_context(tc.tile_pool(name="lhsT", bufs=4))
        psA_pool = ctx.enter_context(
            tc.tile_pool(name="psA", bufs=3, space="PSUM"))
        psU_pool = ctx.enter_context(
            tc.tile_pool(name="psU", bufs=3, space="PSUM"))
        scrap_pool = ctx.enter_context(tc.tile_pool(name="scrap", bufs=3))

        # fp32r-consumed operands must be written by a rounding producer
        xt4r = consts.tile([64, 256], f32r, name="xt4r")
        nc.vector.tensor_copy(xt4r, sb['XT4'])
        cpmr = consts.tile([32, 256], f32r, name="cpmr")
        nc.gpsimd.tensor_copy(cpmr, sb['cpm'])
        i32r = consts.tile([32, 128], f32r, name="i32r")
        nc.vector.tensor_copy(i32r, sb['I32r4'])
        wb0r = consts.tile([128, 128], f32r, name="wb0r")
        nc.gpsimd.tensor_copy(wb0r, sb['WB0'])
        wb1r = consts.tile([128, 128], f32r, name="wb1r")
        nc.vector.tensor_copy(wb1r, sb['WB1'])
        rhsA = xt4r
        rhsC = cpmr
        lhsC = i32r

        # ---- Phase A: H and H^T, one 4-row group (x all 32 channels) per g
        for g in range(G):
            lhsTg = lhsT_pool.tile([64, 128], f32r, name="lhsTg")
            nc.gpsimd.tensor_scalar_mul(lhsTg, sb['WtBD'], sb['Xr'][:, g:g + 1])
            ps = psA_pool.tile([128, N], f32, name="psA")
            nc.tensor.matmul(ps, lhsT=lhsTg, rhs=rhsA,
                             start=True, stop=False, skip_group_check=True)
            hsl = H4[:, g * N:(g + 1) * N]
            nc.scalar.activation(out=hsl, in_=ps, func=AF.Relu,
                                 bias=sb['Cpp'][:, g:g + 1],
                                 accum_out=r4[:, g:g + 1])
            nc.tensor.matmul(ps, lhsT=lhsC, rhs=rhsC,
                             start=False, stop=True, skip_group_check=True)
            tsl = HT4[:, g * N:(g + 1) * N]
            nc.vector.tensor_scalar_max(tsl, ps, 0.0)

        # ---- Small phase: diagonal side computations + rho/kappa biases
        # All small psums rotate through one 2-buf pool (PSUM = 8 banks total;
        # every PSUM tile occupies a full bank).
        small = ctx.enter_context(tc.tile_pool(name="small", bufs=1))
        psm_pool = ctx.enter_context(
            tc.tile_pool(name="psm", bufs=2, space="PSUM"))

        def psm():
            return psm_pool.tile([128, 256], f32, name="psm")

        psS = psm()[0:32, :]
        nc.tensor.matmul(psS, lhsT=sb['wt16'], rhs=sb['X2T'],
                         start=True, stop=True)
        t0 = small.tile([32, 256], f32, name="t0")
        nc.vector.tensor_add(t0, psS, sb['cpm'])
        dn = small.tile([32, 256], f32, name="dn")
        nc.gpsimd.tensor_scalar_max(dn, t0, 0.0)
        t1 = small.tile([32, 256], f32, name="t1")
        nc.vector.tensor_add(t1, t0, sb['abias'])
        dg = small.tile([32, 256], f32, name="dg")
        nc.gpsimd.tensor_scalar_max(dg, t1, 0.0)
        hdc = small.tile([32, 256], f32, name="hdc")
        nc.vector.tensor_sub(hdc, dg, dn)

        hdc4 = small.tile([128, G], f32, name="hdc4")
        hdc_r = hdc.rearrange("s (g a) -> s a g", a=A)
        for a in range(A):
            nc.default_dma_engine.dma_start(
                out=hdc4[a * NH:(a + 1) * NH, :], in_=hdc_r[:, a, :])
        r4hat = small.tile([128, G], f32, name="r4hat")
        nc.vector.tensor_add(r4hat, r4, hdc4)

        rsum = small.tile([128, 1], f32, name="rsum")
        nc.vector.tensor_reduce(out=rsum, in_=r4hat,
                                axis=mybir.AxisListType.X, op=ALU.add)
        psT = psm()[0:32, 0:1]
        nc.tensor.matmul(psT, lhsT=sb['P32'], rhs=rsum, start=True, stop=True)
        Tsb = small.tile([32, 1], f32, name="Tsb")
        nc.scalar.copy(Tsb, psT)
        psK = psm()[0:32, 0:1]
        nc.tensor.matmul(psK, lhsT=sb['W24'], rhs=Tsb, start=True, stop=True)
        ksb = small.tile([32, 1], f32, name="ksb")
        nc.scalar.activation(out=ksb, in_=psK, func=AF.Identity,
                             bias=sb['b2c'])
        psKr = psm()[:, 0:1]
        nc.tensor.matmul(psKr, lhsT=sb['I32r4'], rhs=ksb,
                         start=True, stop=True)
        krep = small.tile([128, 1], f32, name="krep")
        nc.scalar.copy(krep, psKr)

        psR = psm()[:, 0:G]
        nc.tensor.matmul(psR, lhsT=sb['WB3'], rhs=r4hat,
                         start=True, stop=True)
        rhoka = small.tile([128, G], f32, name="rhoka")
        nc.scalar.activation(out=rhoka, in_=psR, func=AF.Identity, bias=krep)

        rhokati = small.tile([32, 256], f32, name="rhokati")
        rt_r = rhokati.rearrange("t (g a) -> t a g", a=A)
        for a in range(A):
            nc.default_dma_engine.dma_start(
                out=rt_r[:, a, :], in_=rhoka[a * NH:(a + 1) * NH, :])

        psQ = psm()[0:32, :]
        nc.tensor.matmul(psQ, lhsT=sb['W01'], rhs=hdc,
                         start=True, stop=False)
        nc.tensor.matmul(psQ, lhsT=sb['W22'], rhs=dg,
                         start=False, stop=True)
        psU2 = psm()[0:32, :]
        nc.tensor.matmul(psU2, lhsT=sb['W01'], rhs=dn,
                         start=True, stop=True)
        uii = small.tile([32, 256], f32, name="uii")
        nc.vector.tensor_add(uii, psU2, rhokati)
        t3 = small.tile([32, 256], f32, name="t3")
        nc.vector.tensor_add(t3, uii, psQ)
        scrapS = small.tile([32, 256], f32, name="scrapS")
        cA = small.tile([32, 1], f32, name="cA")
        nc.vector.tensor_scalar(scrapS, t3, 0.0, None, ALU.max, ALU.add,
                                accum_out=cA)
        scrapS2 = small.tile([32, 256], f32, name="scrapS2")
        cB = small.tile([32, 1], f32, name="cB")
        nc.vector.tensor_scalar(scrapS2, uii, 0.0, None, ALU.max, ALU.add,
                                accum_out=cB)
        corr = small.tile([32, 1], f32, name="corr")
        nc.vector.tensor_sub(corr, cA, cB)

        # ---- Phase B: channel mix + fused bias-relu-rowsum
        lhsM0 = wb0r
        lhsM1 = wb1r
        for g in range(G):
            ps = psU_pool.tile([128, N], f32, name="psU")
            sl = slice(g * N, (g + 1) * N)
            nc.tensor.matmul(ps, lhsT=lhsM0, rhs=H4[:, sl],
                             start=True, stop=False)
            nc.tensor.matmul(ps, lhsT=lhsM1, rhs=HT4[:, sl],
                             start=False, stop=True)
            scrap = scrap_pool.tile([128, N], f32, name="scrap")
            if g % 2 == 0:
                nc.scalar.activation(out=scrap, in_=ps, func=AF.Relu,
                                     bias=rhoka[:, g:g + 1],
                                     accum_out=acc[:, g:g + 1])
            else:
                nc.vector.scalar_tensor_tensor(
                    scrap, ps, rhoka[:, g:g + 1], zero256,
                    ALU.add, ALU.max, accum_out=acc[:, g:g + 1])

        # ---- Pooling + MLP head
        accred = small.tile([128, 1], f32, name="accred")
        nc.vector.tensor_reduce(out=accred, in_=acc,
                                axis=mybir.AxisListType.X, op=ALU.add)
        psP = psm()[0:32, 0:1]
        nc.tensor.matmul(psP, lhsT=sb['P32'], rhs=accred,
                         start=True, stop=True)
        p_sb = small.tile([32, 1], f32, name="p_sb")
        nc.scalar.activation(out=p_sb, in_=psP, func=AF.Relu, bias=corr)
        psY1 = psm()[:, 0:1]
        nc.tensor.matmul(psY1, lhsT=sb['D1m'], rhs=p_sb,
                         start=True, stop=True)
        y1 = small.tile([128, 1], f32, name="y1")
        nc.scalar.activation(out=y1, in_=psY1, func=AF.Relu, bias=sb['db1m'])
        psY2 = psm()[:, 0:1]
        nc.tensor.matmul(psY2, lhsT=sb['D2m'], rhs=y1, start=True, stop=True)
        y2 = small.tile([128, 1], f32, name="y2")
        nc.scalar.activation(out=y2, in_=psY2, func=AF.Relu, bias=sb['db2m'])
        psY3 = psm()[0:1, 0:1]
        nc.tensor.matmul(psY3, lhsT=sb['D3m'], rhs=y2, start=True, stop=True)
        yo = small.tile([1, 1], f32, name="yo")
        nc.scalar.activation(out=yo, in_=psY3, func=AF.Identity,
                             bias=sb['db3m'])
        nc.default_dma_engine.dma_start(out=yout_d.ap(), in_=yo)

        ctx.close()

    nc.compile()
    _PROG_CACHE['nc'] = nc
    return nc


def make_in_maps(inputs):
    x = np.asarray(inputs['x'], dtype=F32)
    args = [np.asarray(inputs[k], dtype=np.float64) for k in
            ('W1', 'b1', 'W2', 'b2', 'D1', 'db1', 'D2', 'db2', 'D3', 'db3')]
    return [_percore_inputs(x[b], *args) for b in range(B)]


def kernel(**inputs) -> np.ndarray:
    from concourse.bass_utils import run_bass_kernel_spmd
    nc = build_program()
    in_maps = make_in_maps(inputs)
    res = run_bass_kernel_spmd(nc, in_maps, core_ids=list(range(B))).results
    return np.concatenate([res[b]['yout'].reshape(1, 1) for b in range(B)],
                          axis=0).astype(F32)


# revision 10
# speedup vs baseline: 1.3547x; 1.3547x over previous
"""Trainium2 Bass kernel for MiniEq2Net (gnn_message_passing).

Math (validated against the jax reference in float64, rel err ~3e-7):

Per batch b (X = x[b], [n=256, d=16]) the first eq-layer's input channels are
diag(X[:,d]) and X[:,d] outer X[:,d], so layer 1 collapses to
    G1[s] = S(s) + c'_{s,i} (row-broadcast) + delta_ij a_{s,i}
with S(s) = X diag(wt_s) X^T (symmetric, one K=64 matmul per 4-row group in a
packed (a=i%4, s) x (j) layout), and the diagonal handled exactly via tiny
[32,256] side computations (dn/dg/Hdc).  Layer 2 + pooling becomes two K=128
block-diagonal channel-mix matmuls over relu'd H and H^T plus a fused
relu-accumulate, with the diagonal / rowsum / total-sum basis terms folded
into per-partition biases and a closed-form correction.

Sharding: pure data parallel, one batch element per NeuronCore (B=8, 8 cores).
All heavy intermediates (H, H^T: 16MB) stay in SBUF; HBM traffic is ~0.6MB of
host-precomputed small operands per core, packed into 3 blob tensors so the
input load is 3 large DMAs instead of ~22 small ones.
"""

import numpy as np

N = 256          # n (graph nodes)
D = 16           # input channel count
NH = 32          # hidden channels
A = 4            # row-packing factor: partition p = a*32+s, row i = 4*g+a
G = N // A       # 64 row-groups
B = 8            # batch == cores
F32 = np.float32

_PROG_CACHE = {}


def _reorder_ag(arr):
    """Permute the trailing i axis (len 256) into (a, g) order:
    out[..., a*G+g] = arr[..., 4*g+a]."""
    sh = arr.shape[:-1]
    return arr.reshape(*sh, G, A).swapaxes(-1, -2).reshape(*sh, N)


# Blob packing: blob_name -> (partition_count, [(tensor_name, P, F), ...])
_BLOBS = {
    'blob128': (128, [
        ('Cpp', 128, G), ('WB0', 128, 128), ('WB1', 128, 128),
        ('WB3', 128, 128), ('P32', 128, 32), ('D2m', 128, 128),
        ('db1m', 128, 1), ('db2m', 128, 1), ('D3m', 128, 1),
    ]),
    'blob64': (64, [
        ('XT4', 64, 256), ('WtBD', 64, 128), ('Xr', 64, G),
    ]),
    'blob32': (32, [
        ('cpm2', 32, 512), ('I32r4', 32, 128),
        ('W01', 32, 32), ('W22', 32, 32), ('W24', 32, 32),
        ('wt16', 16, 32), ('X2Tr', 16, 256), ('cpr', 32, 256),
        ('abiasr', 32, 256), ('b2c', 32, 1), ('D1m', 32, 128),
        ('db3m', 1, 1),
    ]),
}


def _blob_layout():
    where, shapes = {}, {}
    for bname, (pb, items) in _BLOBS.items():
        off = 0
        for tname, p, f in items:
            where[tname] = (bname, p, off, f)
            off += f
        shapes[bname] = (pb, off)
    return where, shapes


_WHERE, _BLOB_SHAPES = _blob_layout()


# ---------------------------------------------------------------- host side

def _percore_inputs(xb, W1, b1, W2, b2, D1, db1, D2, db2, D3, db3):
    """Small per-core operands, precomputed in float64, packed into blobs."""
    X = xb.astype(np.float64)                      # [256, 16]
    n = float(N)
    sigma = X.sum(0)
    wt = W1[D:, :, 0] + W1[D:, :, 1]               # [16,32]
    alpha = W1[:D, :, 0] + W1[:D, :, 1] + W1[:D, :, 2]
    beta = W1[D:, :, 2]
    abias = alpha.T @ X.T + beta.T @ (X.T ** 2)    # [32,256]
    gamma = W1[:D, :, 3] / n + W1[D:, :, 3] * sigma[:, None] / n
    k = (W1[:D, :, 4].T @ (sigma / n**2)
         + W1[D:, :, 4].T @ (sigma**2 / n**2) + b1)
    cp = gamma.T @ X.T + k[:, None]                # [32,256]
    XT = X.T

    WtBD = np.zeros((A * D, 128))
    for a in range(A):
        WtBD[a * D:(a + 1) * D, a * NH:(a + 1) * NH] = wt
    Xr = X.reshape(G, A, D).transpose(1, 2, 0).reshape(A * D, G)
    Cpp = cp.reshape(NH, G, A).transpose(2, 0, 1).reshape(128, G)

    def blockdiag(M):
        out = np.zeros((128, 128))
        for a in range(A):
            out[a * NH:(a + 1) * NH, a * NH:(a + 1) * NH] = M
        return out

    vals = {
        'XT4': np.tile(XT, (A, 1)),
        'cpm2': np.tile(cp, (1, 2)),
        'WtBD': WtBD, 'Xr': Xr,
        'I32r4': np.tile(np.eye(NH), (1, A)),
        'Cpp': Cpp,
        'WB0': blockdiag(W2[:, :, 0]), 'WB1': blockdiag(W2[:, :, 1]),
        'WB3': blockdiag(W2[:, :, 3] / n),
        'P32': np.tile(np.eye(NH), (A, 1)),
        'W01': W2[:, :, 0] + W2[:, :, 1], 'W22': W2[:, :, 2],
        'W24': W2[:, :, 4] / n**2,
        'wt16': wt,
        'X2Tr': _reorder_ag(XT ** 2),
        'cpr': _reorder_ag(cp),
        'abiasr': _reorder_ag(abias),
        'b2c': b2[:, None],
        'D1m': D1, 'db1m': db1[:, None],
        'D2m': D2, 'db2m': db2[:, None],
        'D3m': D3, 'db3m': db3[:, None],
    }
    blobs = {bn: np.zeros(sh, dtype=F32) for bn, sh in _BLOB_SHAPES.items()}
    for tname, (bn, p, off, f) in _WHERE.items():
        v = np.asarray(vals[tname], dtype=np.float64)
        assert v.shape == (p, f), (tname, v.shape, (p, f))
        blobs[bn][0:p, off:off + f] = v.astype(F32)
    return blobs


# -------------------------------------------------------------- device side

def build_program():
    if 'nc' in _PROG_CACHE:
        return _PROG_CACHE['nc']

    from contextlib import ExitStack
    import concourse.bacc as bacc
    import concourse.tile as tile
    from concourse import mybir

    f32 = mybir.dt.float32
    f32r = mybir.dt.float32r
    AF = mybir.ActivationFunctionType
    ALU = mybir.AluOpType

    nc = bacc.Bacc(trn_type="TRN2", target_bir_lowering=False)
    dram = {bn: nc.dram_tensor(bn, list(sh), f32, kind="ExternalInput")
            for bn, sh in _BLOB_SHAPES.items()}
    yout_d = nc.dram_tensor("yout", [1, 1], f32, kind="ExternalOutput")

    with tile.TileContext(nc) as tc:
        ctx = ExitStack()
        consts = ctx.enter_context(tc.tile_pool(name="consts", bufs=1))
        bt = {}
        for bn, sh in _BLOB_SHAPES.items():
            t = consts.tile(list(sh), f32, name=f"sb_{bn}")
            nc.default_dma_engine.dma_start(out=t, in_=dram[bn].ap())
            bt[bn] = t
        sb = {tn: bt[bn][0:p, off:off + f]
              for tn, (bn, p, off, f) in _WHERE.items()}

        big = ctx.enter_context(tc.tile_pool(name="big", bufs=1))
        zero256 = big.tile([128, 256], f32, name="zero256")
        nc.vector.memset(zero256, 0.0)
        H4 = big.tile([128, G * N], f32r, name="H4")
        HT4 = big.tile([128, G * N], f32r, name="HT4")
        r4 = big.tile([128, G], f32, name="r4")
        acc = big.tile([128, G], f32, name="acc")

        lhsT_pool = ctx.enter_context(tc.tile_pool(name="lhsT", bufs=4))
        psA_pool = ctx.enter_context(
            tc.tile_pool(name="psA", bufs=3, space="PSUM"))
        psU_pool = ctx.enter_context(
            tc.tile_pool(name="psU", bufs=3, space="PSUM"))
        scrap_pool = ctx.enter_context(tc.tile_pool(name="scrap", bufs=3))
        small = ctx.enter_context(tc.tile_pool(name="small", bufs=1))
        psm_pool = ctx.enter_context(
            tc.tile_pool(name="psm", bufs=2, space="PSUM"))

        def psm():
            return psm_pool.tile([128, 512], f32, name="psm")

        # fp32r-consumed operands need a rounding producer
        xt4r = consts.tile([64, 256], f32r, name="xt4r")
        nc.gpsimd.tensor_copy(xt4r, sb['XT4'])
        cpm2r = consts.tile([32, 512], f32r, name="cpm2r")
        nc.gpsimd.tensor_copy(cpm2r, sb['cpm2'])
        i32r = consts.tile([32, 128], f32r, name="i32r")
        nc.gpsimd.tensor_copy(i32r, sb['I32r4'])
        wb0r = consts.tile([128, 128], f32r, name="wb0r")
        nc.gpsimd.tensor_copy(wb0r, sb['WB0'])
        wb1r = consts.tile([128, 128], f32r, name="wb1r")
        nc.gpsimd.tensor_copy(wb1r, sb['WB1'])

        # ---- Small-phase prefix (independent of H; overlaps phase A).
        # All [32, 256] tensors here use (a, g) column order: column a*G+g
        # holds logical row i = 4g+a, so the [32,256] -> [128,64] repack is
        # 4 contiguous DMAs.
        psS = psm()[0:32, 0:256]
        nc.tensor.matmul(psS, lhsT=sb['wt16'], rhs=sb['X2Tr'],
                         start=True, stop=True, skip_group_check=True)
        t0 = small.tile([32, 256], f32, name="t0")
        nc.vector.tensor_add(t0, psS, sb['cpr'])
        dn = small.tile([32, 256], f32, name="dn")
        nc.gpsimd.tensor_scalar_max(dn, t0, 0.0)
        t1 = small.tile([32, 256], f32, name="t1")
        nc.vector.tensor_add(t1, t0, sb['abiasr'])
        dg = small.tile([32, 256], f32, name="dg")
        nc.gpsimd.tensor_scalar_max(dg, t1, 0.0)
        hdc = small.tile([32, 256], f32, name="hdc")
        nc.vector.tensor_sub(hdc, dg, dn)
        hdc4 = small.tile([128, G], f32, name="hdc4")
        for a in range(A):
            nc.default_dma_engine.dma_start(
                out=hdc4[a * NH:(a + 1) * NH, :],
                in_=hdc[:, a * G:(a + 1) * G])
        psQ = psm()[0:32, 0:256]
        nc.tensor.matmul(psQ, lhsT=sb['W01'], rhs=hdc,
                         start=True, stop=False, skip_group_check=True)
        nc.tensor.matmul(psQ, lhsT=sb['W22'], rhs=dg,
                         start=False, stop=True, skip_group_check=True)
        qsb = small.tile([32, 256], f32, name="qsb")
        nc.scalar.copy(qsb, psQ)
        psU2 = psm()[0:32, 0:256]
        nc.tensor.matmul(psU2, lhsT=sb['W01'], rhs=dn,
                         start=True, stop=True, skip_group_check=True)
        u2sb = small.tile([32, 256], f32, name="u2sb")
        nc.scalar.copy(u2sb, psU2)

        # ---- Phase A: H and H^T tiles; 2 row-groups share one PSUM bank
        for c in range(G // 2):
            g0, g1 = 2 * c, 2 * c + 1
            ps = psA_pool.tile([128, 512], f32, name="psA")
            for k, g in ((0, g0), (1, g1)):
                lhsTg = lhsT_pool.tile([64, 128], f32r, name="lhsTg")
                nc.gpsimd.tensor_scalar_mul(lhsTg, sb['WtBD'],
                                            sb['Xr'][:, g:g + 1])
                half = ps[:, k * N:(k + 1) * N]
                # start=True zeroes the whole 2KB PSUM zero-region (bank),
                # so only the first matmul in this bank may set it; the
                # second half is zeroed by its own first write (pending).
                nc.tensor.matmul(half, lhsT=lhsTg, rhs=xt4r,
                                 start=(k == 0), stop=False,
                                 skip_group_check=True)
                # H = relu(S + c'_i) (row bias per partition); row sums
                # accumulate into r4 for the later rho/kappa biases.
                nc.vector.scalar_tensor_tensor(
                    H4[:, g * N:(g + 1) * N], half, sb['Cpp'][:, g:g + 1],
                    zero256, ALU.add, ALU.max, accum_out=r4[:, g:g + 1])
            # S + c'_j for both halves in one K=32 matmul, one wide relu
            nc.tensor.matmul(ps, lhsT=i32r, rhs=cpm2r,
                             start=False, stop=True, skip_group_check=True)
            nc.scalar.activation(out=HT4[:, g0 * N:(g1 + 1) * N], in_=ps,
                                 func=AF.Relu)

        # ---- Small-phase suffix: rho/kappa biases (needs all of r4)
        r4hat = small.tile([128, G], f32, name="r4hat")
        nc.vector.tensor_add(r4hat, r4, hdc4)
        rsum = small.tile([128, 1], f32, name="rsum")
        nc.vector.tensor_reduce(out=rsum, in_=r4hat,
                                axis=mybir.AxisListType.X, op=ALU.add)
        psT = psm()
        nc.tensor.matmul(psT[0:32, 0:1], lhsT=sb['P32'], rhs=rsum,
                         start=True, stop=True, skip_group_check=True)
        Tsb = small.tile([32, 1], f32, name="Tsb")
        nc.scalar.copy(Tsb, psT[0:32, 0:1])
        nc.tensor.matmul(psT[0:32, 4:5], lhsT=sb['W24'], rhs=Tsb,
                         start=True, stop=True, skip_group_check=True)
        ksb = small.tile([32, 1], f32, name="ksb")
        nc.scalar.activation(out=ksb, in_=psT[0:32, 4:5], func=AF.Identity,
                             bias=sb['b2c'])
        nc.tensor.matmul(psT[:, 8:9], lhsT=sb['I32r4'], rhs=ksb,
                         start=True, stop=True, skip_group_check=True)
        krep = small.tile([128, 1], f32, name="krep")
        nc.scalar.copy(krep, psT[:, 8:9])
        nc.tensor.matmul(psT[:, 64:64 + G], lhsT=sb['WB3'], rhs=r4hat,
                         start=True, stop=True, skip_group_check=True)
        rhoka = small.tile([128, G], f32, name="rhoka")
        nc.scalar.activation(out=rhoka, in_=psT[:, 64:64 + G],
                             func=AF.Identity, bias=krep)

        # corr path ((a,g) order throughout) — runs parallel with phase B
        rhokr = small.tile([32, 256], f32, name="rhokr")
        for a in range(A):
            nc.default_dma_engine.dma_start(
                out=rhokr[:, a * G:(a + 1) * G],
                in_=rhoka[a * NH:(a + 1) * NH, :])
        uii = small.tile([32, 256], f32, name="uii")
        nc.vector.tensor_add(uii, u2sb, rhokr)
        t3 = small.tile([32, 256], f32, name="t3")
        nc.vector.tensor_add(t3, uii, qsb)
        scrapS = small.tile([32, 256], f32, name="scrapS")
        cA = small.tile([32, 1], f32, name="cA")
        nc.vector.tensor_scalar(scrapS, t3, 0.0, None, ALU.max, ALU.add,
                                accum_out=cA)
        scrapS2 = small.tile([32, 256], f32, name="scrapS2")
        cB = small.tile([32, 1], f32, name="cB")
        nc.vector.tensor_scalar(scrapS2, uii, 0.0, None, ALU.max, ALU.add,
                                accum_out=cB)
        corr = small.tile([32, 1], f32, name="corr")
        nc.vector.tensor_sub(corr, cA, cB)

        # ---- Phase B: channel mix + fused bias-relu-rowsum
        # DVE is cheaper per op; Act takes the share DVE can't absorb.
        for g in range(G):
            ps = psU_pool.tile([128, N], f32, name="psU")
            sl = slice(g * N, (g + 1) * N)
            nc.tensor.matmul(ps, lhsT=wb0r, rhs=H4[:, sl],
                             start=True, stop=False)
            nc.tensor.matmul(ps, lhsT=wb1r, rhs=HT4[:, sl],
                             start=False, stop=True)
            scrap = scrap_pool.tile([128, N], f32, name="scrap")
            if g % 2 == 0:
                nc.vector.scalar_tensor_tensor(
                    scrap, ps, rhoka[:, g:g + 1], zero256,
                    ALU.add, ALU.max, accum_out=acc[:, g:g + 1])
            else:
                nc.scalar.activation(out=scrap, in_=ps, func=AF.Relu,
                                     bias=rhoka[:, g:g + 1],
                                     accum_out=acc[:, g:g + 1])

        # ---- Pooling + MLP head
        accred = small.tile([128, 1], f32, name="accred")
        nc.vector.tensor_reduce(out=accred, in_=acc,
                                axis=mybir.AxisListType.X, op=ALU.add)
        psY = psm()
        nc.tensor.matmul(psY[0:32, 0:1], lhsT=sb['P32'], rhs=accred,
                         start=True, stop=True, skip_group_check=True)
        p_sb = small.tile([32, 1], f32, name="p_sb")
        nc.scalar.activation(out=p_sb, in_=psY[0:32, 0:1], func=AF.Relu,
                             bias=corr)
        nc.tensor.matmul(psY[:, 4:5], lhsT=sb['D1m'], rhs=p_sb,
                         start=True, stop=True, skip_group_check=True)
        y1 = small.tile([128, 1], f32, name="y1")
        nc.scalar.activation(out=y1, in_=psY[:, 4:5], func=AF.Relu,
                             bias=sb['db1m'])
        nc.tensor.matmul(psY[:, 8:9], lhsT=sb['D2m'], rhs=y1,
                         start=True, stop=True, skip_group_check=True)
        y2 = small.tile([128, 1], f32, name="y2")
        nc.scalar.activation(out=y2, in_=psY[:, 8:9], func=AF.Relu,
                             bias=sb['db2m'])
        nc.tensor.matmul(psY[0:1, 12:13], lhsT=sb['D3m'], rhs=y2,
                         start=True, stop=True, skip_group_check=True)
        yo = small.tile([1, 1], f32, name="yo")
        nc.scalar.activation(out=yo, in_=psY[0:1, 12:13], func=AF.Identity,
                             bias=sb['db3m'])
        nc.default_dma_engine.dma_start(out=yout_d.ap(), in_=yo)

        ctx.close()

    nc.compile()
    _PROG_CACHE['nc'] = nc
    return nc


def make_in_maps(inputs):
    x = np.asarray(inputs['x'], dtype=F32)
    args = [np.asarray(inputs[k], dtype=np.float64) for k in
            ('W1', 'b1', 'W2', 'b2', 'D1', 'db1', 'D2', 'db2', 'D3', 'db3')]
    return [_percore_inputs(x[b], *args) for b in range(B)]


def kernel(**inputs) -> np.ndarray:
    from concourse.bass_utils import run_bass_kernel_spmd
    nc = build_program()
    in_maps = make_in_maps(inputs)
    res = run_bass_kernel_spmd(nc, in_maps, core_ids=list(range(B))).results
    return np.concatenate([res[b]['yout'].reshape(1, 1) for b in range(B)],
                          axis=0).astype(F32)


# revision 17
# speedup vs baseline: 1.4398x; 1.0628x over previous
"""Trainium2 Bass kernel for MiniEq2Net (gnn_message_passing).

Math (validated against the jax reference in float64, rel err ~3e-7):

Per batch b (X = x[b], [n=256, d=16]) the first eq-layer's input channels are
diag(X[:,d]) and X[:,d] outer X[:,d], so layer 1 collapses to
    G1[s] = S(s) + c'_{s,i} (row-broadcast) + delta_ij a_{s,i}
with S(s) = X diag(wt_s) X^T (symmetric, one K=64 matmul per 4-row group in a
packed (a=i%4, s) x (j) layout), and the diagonal handled exactly via tiny
[32,256] side computations (dn/dg/Hdc).  Layer 2 + pooling becomes two K=128
block-diagonal channel-mix matmuls over relu'd H and H^T plus a fused
relu-accumulate, with the diagonal / rowsum / total-sum basis terms folded
into per-partition biases and a closed-form correction.

Sharding: pure data parallel, one batch element per NeuronCore (B=8, 8 cores).
All heavy intermediates (H, H^T: 16MB) stay in SBUF; HBM traffic is ~0.6MB of
host-precomputed small operands per core, packed into 3 blob tensors so the
input load is 3 large DMAs instead of ~22 small ones.
"""

import numpy as np

N = 256          # n (graph nodes)
D = 16           # input channel count
NH = 32          # hidden channels
A = 4            # row-packing factor: partition p = a*32+s, row i = 4*g+a
G = N // A       # 64 row-groups
B = 8            # batch == cores
F32 = np.float32

_PROG_CACHE = {}


def _reorder_ag(arr):
    """Permute the trailing i axis (len 256) into (a, g) order:
    out[..., a*G+g] = arr[..., 4*g+a]."""
    sh = arr.shape[:-1]
    return arr.reshape(*sh, G, A).swapaxes(-1, -2).reshape(*sh, N)


# Blob packing: blob_name -> (partition_count, [(tensor_name, P, F), ...])
_BLOBS = {
    'blob128': (128, [
        ('Cpp', 128, G), ('WB0', 128, 128), ('WB1', 128, 128),
        ('WB3', 128, 128), ('P32', 128, 32), ('D2m', 128, 128),
        ('db1m', 128, 1), ('db2m', 128, 1), ('D3m', 128, 1),
        ('PW', 128, 32),
    ]),
    'blob64': (64, [
        ('XT4', 64, 256), ('WtBD', 64, 128), ('Xr', 64, G),
    ]),
    'blob32': (32, [
        ('cpm2', 32, 512), ('I32r4', 32, 128),
        ('W01', 32, 32), ('W22', 32, 32), ('W24', 32, 32),
        ('wt16', 16, 32), ('X2Tr', 16, 256), ('cpr', 32, 256),
        ('abiasr', 32, 256), ('b2c', 32, 1), ('D1m', 32, 128),
        ('db3m', 1, 1),
    ]),
}


def _blob_layout():
    where, shapes = {}, {}
    for bname, (pb, items) in _BLOBS.items():
        off = 0
        for tname, p, f in items:
            where[tname] = (bname, p, off, f)
            off += f
        shapes[bname] = (pb, off)
    return where, shapes


_WHERE, _BLOB_SHAPES = _blob_layout()


# ---------------------------------------------------------------- host side

def _percore_inputs(xb, W1, b1, W2, b2, D1, db1, D2, db2, D3, db3):
    """Small per-core operands, precomputed in float64, packed into blobs."""
    X = xb.astype(np.float64)                      # [256, 16]
    n = float(N)
    sigma = X.sum(0)
    wt = W1[D:, :, 0] + W1[D:, :, 1]               # [16,32]
    alpha = W1[:D, :, 0] + W1[:D, :, 1] + W1[:D, :, 2]
    beta = W1[D:, :, 2]
    abias = alpha.T @ X.T + beta.T @ (X.T ** 2)    # [32,256]
    gamma = W1[:D, :, 3] / n + W1[D:, :, 3] * sigma[:, None] / n
    k = (W1[:D, :, 4].T @ (sigma / n**2)
         + W1[D:, :, 4].T @ (sigma**2 / n**2) + b1)
    cp = gamma.T @ X.T + k[:, None]                # [32,256]
    XT = X.T

    WtBD = np.zeros((A * D, 128))
    for a in range(A):
        WtBD[a * D:(a + 1) * D, a * NH:(a + 1) * NH] = wt
    Xr = X.reshape(G, A, D).transpose(1, 2, 0).reshape(A * D, G)
    Cpp = cp.reshape(NH, G, A).transpose(2, 0, 1).reshape(128, G)

    def blockdiag(M):
        out = np.zeros((128, 128))
        for a in range(A):
            out[a * NH:(a + 1) * NH, a * NH:(a + 1) * NH] = M
        return out

    vals = {
        'XT4': np.tile(XT, (A, 1)),
        'cpm2': np.tile(cp, (1, 2)),
        'WtBD': WtBD, 'Xr': Xr,
        'I32r4': np.tile(np.eye(NH), (1, A)),
        'Cpp': Cpp,
        'WB0': blockdiag(W2[:, :, 0]), 'WB1': blockdiag(W2[:, :, 1]),
        'WB3': blockdiag(W2[:, :, 3] / n),
        'P32': np.tile(np.eye(NH), (A, 1)),
        'PW': np.tile(np.eye(NH), (A, 1)) @ (W2[:, :, 4] / n**2),
        'W01': W2[:, :, 0] + W2[:, :, 1], 'W22': W2[:, :, 2],
        'W24': W2[:, :, 4] / n**2,
        'wt16': wt,
        'X2Tr': _reorder_ag(XT ** 2),
        'cpr': _reorder_ag(cp),
        'abiasr': _reorder_ag(abias),
        'b2c': b2[:, None],
        'D1m': D1, 'db1m': db1[:, None],
        'D2m': D2, 'db2m': db2[:, None],
        'D3m': D3, 'db3m': db3[:, None],
    }
    blobs = {bn: np.zeros(sh, dtype=F32) for bn, sh in _BLOB_SHAPES.items()}
    for tname, (bn, p, off, f) in _WHERE.items():
        v = np.asarray(vals[tname], dtype=np.float64)
        assert v.shape == (p, f), (tname, v.shape, (p, f))
        blobs[bn][0:p, off:off + f] = v.astype(F32)
    return blobs


# -------------------------------------------------------------- device side

def build_program():
    if 'nc' in _PROG_CACHE:
        return _PROG_CACHE['nc']

    from contextlib import ExitStack
    import concourse.bacc as bacc
    import concourse.tile as tile
    from concourse import mybir

    f32 = mybir.dt.float32
    f32r = mybir.dt.float32r
    bf16 = mybir.dt.bfloat16
    AF = mybir.ActivationFunctionType
    ALU = mybir.AluOpType

    nc = bacc.Bacc(trn_type="TRN2", target_bir_lowering=False)
    dram = {bn: nc.dram_tensor(bn, list(sh), f32, kind="ExternalInput")
            for bn, sh in _BLOB_SHAPES.items()}
    yout_d = nc.dram_tensor("yout", [1, 1], f32, kind="ExternalOutput")

    with tile.TileContext(nc) as tc:
        ctx = ExitStack()
        consts = ctx.enter_context(tc.tile_pool(name="consts", bufs=1))
        bt = {}
        for bn, sh in _BLOB_SHAPES.items():
            t = consts.tile(list(sh), f32, name=f"sb_{bn}")
            nc.default_dma_engine.dma_start(out=t, in_=dram[bn].ap())
            bt[bn] = t
        sb = {tn: bt[bn][0:p, off:off + f]
              for tn, (bn, p, off, f) in _WHERE.items()}

        big = ctx.enter_context(tc.tile_pool(name="big", bufs=1))
        zero256 = big.tile([128, 256], f32, name="zero256")
        nc.vector.memset(zero256, 0.0)
        H4 = big.tile([128, G * N], f32r, name="H4")
        HT4 = big.tile([128, G * N], f32r, name="HT4")
        r4 = big.tile([128, G], f32, name="r4")
        acc = big.tile([128, G], f32, name="acc")

        lhsT_pool = ctx.enter_context(tc.tile_pool(name="lhsT", bufs=4))
        psA_pool = ctx.enter_context(
            tc.tile_pool(name="psA", bufs=4, space="PSUM"))
        psU_pool = ctx.enter_context(
            tc.tile_pool(name="psU", bufs=3, space="PSUM"))
        scrap_pool = ctx.enter_context(tc.tile_pool(name="scrap", bufs=3))
        small = ctx.enter_context(tc.tile_pool(name="small", bufs=1))
        psm_pool = ctx.enter_context(
            tc.tile_pool(name="psm", bufs=1, space="PSUM"))

        def psm():
            return psm_pool.tile([128, 512], f32, name="psm")

        # fp32r-consumed operands need a rounding producer
        xt4r = consts.tile([64, 256], f32r, name="xt4r")
        nc.gpsimd.tensor_copy(xt4r, sb['XT4'])
        cpm2r = consts.tile([32, 512], f32r, name="cpm2r")
        nc.gpsimd.tensor_copy(cpm2r, sb['cpm2'])
        i32r = consts.tile([32, 128], f32r, name="i32r")
        nc.gpsimd.tensor_copy(i32r, sb['I32r4'])
        wb0r = consts.tile([128, 128], f32r, name="wb0r")
        nc.gpsimd.tensor_copy(wb0r, sb['WB0'])
        wb1r = consts.tile([128, 128], f32r, name="wb1r")
        nc.gpsimd.tensor_copy(wb1r, sb['WB1'])

        # ---- Small-phase prefix (independent of H; overlaps phase A).
        # All [32, 256] tensors here use (a, g) column order: column a*G+g
        # holds logical row i = 4g+a, so the [32,256] -> [128,64] repack is
        # 4 contiguous DMAs.
        psS = psm()[0:32, 0:256]
        nc.tensor.matmul(psS, lhsT=sb['wt16'], rhs=sb['X2Tr'],
                         start=True, stop=True, skip_group_check=True)
        t0 = small.tile([32, 256], f32, name="t0")
        nc.vector.tensor_add(t0, psS, sb['cpr'])
        dn = small.tile([32, 256], f32, name="dn")
        nc.gpsimd.tensor_scalar_max(dn, t0, 0.0)
        t1 = small.tile([32, 256], f32, name="t1")
        nc.vector.tensor_add(t1, t0, sb['abiasr'])
        dg = small.tile([32, 256], f32, name="dg")
        nc.gpsimd.tensor_scalar_max(dg, t1, 0.0)
        hdc = small.tile([32, 256], f32, name="hdc")
        nc.vector.tensor_sub(hdc, dg, dn)
        hdc4 = small.tile([128, G], f32, name="hdc4")
        for a in range(A):
            nc.default_dma_engine.dma_start(
                out=hdc4[a * NH:(a + 1) * NH, :],
                in_=hdc[:, a * G:(a + 1) * G])
        psQ = psm()[0:32, 0:256]
        nc.tensor.matmul(psQ, lhsT=sb['W01'], rhs=hdc,
                         start=True, stop=False, skip_group_check=True)
        nc.tensor.matmul(psQ, lhsT=sb['W22'], rhs=dg,
                         start=False, stop=True, skip_group_check=True)
        qsb = small.tile([32, 256], f32, name="qsb")
        nc.scalar.copy(qsb, psQ)
        psU2 = psm()[0:32, 0:256]
        nc.tensor.matmul(psU2, lhsT=sb['W01'], rhs=dn,
                         start=True, stop=True, skip_group_check=True)
        u2sb = small.tile([32, 256], f32, name="u2sb")
        nc.scalar.copy(u2sb, psU2)

        # ---- Phase A: H and H^T tiles; 2 row-groups share one PSUM bank
        for c in range(G // 2):
            g0, g1 = 2 * c, 2 * c + 1
            ps = psA_pool.tile([128, 512], f32, name="psA")
            for k, g in ((0, g0), (1, g1)):
                lhsTg = lhsT_pool.tile([64, 128], f32r, name="lhsTg")
                nc.gpsimd.tensor_scalar_mul(lhsTg, sb['WtBD'],
                                            sb['Xr'][:, g:g + 1])
                half = ps[:, k * N:(k + 1) * N]
                # start=True zeroes the whole 2KB PSUM zero-region (bank),
                # so only the first matmul in this bank may set it; the
                # second half is zeroed by its own first write (pending).
                nc.tensor.matmul(half, lhsT=lhsTg, rhs=xt4r,
                                 start=(k == 0), stop=False,
                                 skip_group_check=True)
                # H = relu(S + c'_i) (row bias per partition); row sums
                # accumulate into r4 for the later rho/kappa biases.
                if g % 16 == 15:
                    nc.scalar.activation(
                        out=H4[:, g * N:(g + 1) * N], in_=half, func=AF.Relu,
                        bias=sb['Cpp'][:, g:g + 1], accum_out=r4[:, g:g + 1])
                else:
                    nc.vector.scalar_tensor_tensor(
                        H4[:, g * N:(g + 1) * N], half, sb['Cpp'][:, g:g + 1],
                        zero256, ALU.add, ALU.max, accum_out=r4[:, g:g + 1])
            # S + c'_j for both halves in one K=32 matmul, one wide relu
            nc.tensor.matmul(ps, lhsT=i32r, rhs=cpm2r,
                             start=False, stop=True, skip_group_check=True)
            nc.scalar.activation(out=HT4[:, g0 * N:(g1 + 1) * N], in_=ps,
                                 func=AF.Relu)

        # ---- Small-phase suffix: rho/kappa biases (needs all of r4)
        r4hat = small.tile([128, G], f32, name="r4hat")
        nc.vector.tensor_add(r4hat, r4, hdc4)
        rsum = small.tile([128, 1], f32, name="rsum")
        nc.vector.tensor_reduce(out=rsum, in_=r4hat,
                                axis=mybir.AxisListType.X, op=ALU.add)
        psT = psm()
        nc.tensor.matmul(psT[0:32, 4:5], lhsT=sb['PW'], rhs=rsum,
                         start=True, stop=True, skip_group_check=True)
        ksb = small.tile([32, 1], f32, name="ksb")
        nc.scalar.activation(out=ksb, in_=psT[0:32, 4:5], func=AF.Identity,
                             bias=sb['b2c'])
        nc.tensor.matmul(psT[:, 8:9], lhsT=sb['I32r4'], rhs=ksb,
                         start=True, stop=True, skip_group_check=True)
        krep = small.tile([128, 1], f32, name="krep")
        nc.scalar.copy(krep, psT[:, 8:9])
        nc.tensor.matmul(psT[:, 64:64 + G], lhsT=sb['WB3'], rhs=r4hat,
                         start=True, stop=True, skip_group_check=True)
        rhoka = small.tile([128, G], f32, name="rhoka")
        nc.scalar.activation(out=rhoka, in_=psT[:, 64:64 + G],
                             func=AF.Identity, bias=krep)

        # corr path ((a,g) order throughout) — runs parallel with phase B
        rhokr = small.tile([32, 256], f32, name="rhokr")
        for a in range(A):
            nc.default_dma_engine.dma_start(
                out=rhokr[:, a * G:(a + 1) * G],
                in_=rhoka[a * NH:(a + 1) * NH, :])
        uii = small.tile([32, 256], f32, name="uii")
        nc.gpsimd.tensor_add(uii, u2sb, rhokr)
        t3 = small.tile([32, 256], f32, name="t3")
        nc.gpsimd.tensor_add(t3, uii, qsb)
        scrapS = small.tile([32, 256], f32, name="scrapS")
        cA = small.tile([32, 1], f32, name="cA")
        nc.vector.tensor_scalar(scrapS, t3, 0.0, None, ALU.max, ALU.add,
                                accum_out=cA)
        scrapS2 = small.tile([32, 256], f32, name="scrapS2")
        cB = small.tile([32, 1], f32, name="cB")
        nc.vector.tensor_scalar(scrapS2, uii, 0.0, None, ALU.max, ALU.add,
                                accum_out=cB)
        corr = small.tile([32, 1], f32, name="corr")
        nc.vector.tensor_sub(corr, cA, cB)

        # ---- Phase B: channel mix + fused bias-relu-rowsum.
        # DVE's fused op is cheaper (392ns vs 585ns exclusive), so it takes
        # the larger share.
        for g in range(G):
            ps = psU_pool.tile([128, N], f32, name="psU")
            sl = slice(g * N, (g + 1) * N)
            nc.tensor.matmul(ps, lhsT=wb0r, rhs=H4[:, sl],
                             start=True, stop=False, skip_group_check=True)
            nc.tensor.matmul(ps, lhsT=wb1r, rhs=HT4[:, sl],
                             start=False, stop=True, skip_group_check=True)
            scrap = scrap_pool.tile([128, N], f32, name="scrap")
            if g % 2 == 0:
                nc.vector.scalar_tensor_tensor(
                    scrap, ps, rhoka[:, g:g + 1], zero256,
                    ALU.add, ALU.max, accum_out=acc[:, g:g + 1])
            else:
                nc.scalar.activation(out=scrap, in_=ps, func=AF.Relu,
                                     bias=rhoka[:, g:g + 1],
                                     accum_out=acc[:, g:g + 1])

        # ---- Pooling + MLP head
        accred = small.tile([128, 1], f32, name="accred")
        nc.vector.tensor_reduce(out=accred, in_=acc,
                                axis=mybir.AxisListType.X, op=ALU.add)
        psY = psm()
        nc.tensor.matmul(psY[0:32, 0:1], lhsT=sb['P32'], rhs=accred,
                         start=True, stop=True, skip_group_check=True)
        p_sb = small.tile([32, 1], f32, name="p_sb")
        nc.scalar.activation(out=p_sb, in_=psY[0:32, 0:1], func=AF.Relu,
                             bias=corr)
        nc.tensor.matmul(psY[:, 4:5], lhsT=sb['D1m'], rhs=p_sb,
                         start=True, stop=True, skip_group_check=True)
        y1 = small.tile([128, 1], f32, name="y1")
        nc.scalar.activation(out=y1, in_=psY[:, 4:5], func=AF.Relu,
                             bias=sb['db1m'])
        nc.tensor.matmul(psY[:, 8:9], lhsT=sb['D2m'], rhs=y1,
                         start=True, stop=True, skip_group_check=True)
        y2 = small.tile([128, 1], f32, name="y2")
        nc.scalar.activation(out=y2, in_=psY[:, 8:9], func=AF.Relu,
                             bias=sb['db2m'])
        nc.tensor.matmul(psY[0:1, 12:13], lhsT=sb['D3m'], rhs=y2,
                         start=True, stop=True, skip_group_check=True)
        yo = small.tile([1, 1], f32, name="yo")
        nc.scalar.activation(out=yo, in_=psY[0:1, 12:13], func=AF.Identity,
                             bias=sb['db3m'])
        nc.default_dma_engine.dma_start(out=yout_d.ap(), in_=yo)

        ctx.close()

    nc.compile()
    _PROG_CACHE['nc'] = nc
    return nc


def make_in_maps(inputs):
    x = np.asarray(inputs['x'], dtype=F32)
    args = [np.asarray(inputs[k], dtype=np.float64) for k in
            ('W1', 'b1', 'W2', 'b2', 'D1', 'db1', 'D2', 'db2', 'D3', 'db3')]
    return [_percore_inputs(x[b], *args) for b in range(B)]


def kernel(**inputs) -> np.ndarray:
    from concourse.bass_utils import run_bass_kernel_spmd
    nc = build_program()
    in_maps = make_in_maps(inputs)
    res = run_bass_kernel_spmd(nc, in_maps, core_ids=list(range(B))).results
    return np.concatenate([res[b]['yout'].reshape(1, 1) for b in range(B)],
                          axis=0).astype(F32)


# revision 19
# speedup vs baseline: 6153.6722x; 4274.1019x over previous
"""Trainium2 Bass kernel for MiniEq2Net (gnn_message_passing).

Math (validated against the jax reference in float64, rel err ~3e-7):

Per batch b (X = x[b], [n=256, d=16]) the first eq-layer's input channels are
diag(X[:,d]) and X[:,d] outer X[:,d], so layer 1 collapses to
    G1[s] = S(s) + c'_{s,i} (row-broadcast) + delta_ij a_{s,i}
with S(s) = X diag(wt_s) X^T (symmetric, one K=64 matmul per 4-row group in a
packed (a=i%4, s) x (j) layout), and the diagonal handled exactly via tiny
[32,256] side computations (dn/dg/Hdc).  Layer 2 + pooling becomes two K=128
block-diagonal channel-mix matmuls over relu'd H and H^T plus a fused
relu-accumulate, with the diagonal / rowsum / total-sum basis terms folded
into per-partition biases and a closed-form correction.

Sharding: pure data parallel, one batch element per NeuronCore (B=8, 8 cores).
All heavy intermediates (H, H^T: 16MB) stay in SBUF; HBM traffic is ~0.6MB of
host-precomputed small operands per core, packed into 3 blob tensors so the
input load is 3 large DMAs instead of ~22 small ones.
"""

import numpy as np

N = 256          # n (graph nodes)
D = 16           # input channel count
NH = 32          # hidden channels
A = 4            # row-packing factor: partition p = a*32+s, row i = 4*g+a
G = N // A       # 64 row-groups
B = 8            # batch == cores
F32 = np.float32

_PROG_CACHE = {}


def _reorder_ag(arr):
    """Permute the trailing i axis (len 256) into (a, g) order:
    out[..., a*G+g] = arr[..., 4*g+a]."""
    sh = arr.shape[:-1]
    return arr.reshape(*sh, G, A).swapaxes(-1, -2).reshape(*sh, N)


# Blob packing: blob_name -> (partition_count, [(tensor_name, P, F), ...])
_BLOBS = {
    'blob128': (128, [
        ('Cpp', 128, G), ('WB0', 128, 128), ('WB1', 128, 128),
        ('WB3', 128, 128), ('P32', 128, 32), ('D2m', 128, 128),
        ('db1m', 128, 1), ('db2m', 128, 1), ('D3m', 128, 1),
        ('PW', 128, 32),
    ]),
    'blob64': (64, [
        ('XT4', 64, 256), ('WtBD', 64, 128), ('Xr', 64, G),
    ]),
    'blob32': (32, [
        ('cpm2', 32, 512), ('I32r4', 32, 128),
        ('W01', 32, 32), ('W22', 32, 32), ('W24', 32, 32),
        ('wt16', 16, 32), ('X2Tr', 16, 256), ('cpr', 32, 256),
        ('abiasr', 32, 256), ('b2c', 32, 1), ('D1m', 32, 128),
        ('db3m', 1, 1),
    ]),
}


def _blob_layout():
    where, shapes = {}, {}
    for bname, (pb, items) in _BLOBS.items():
        off = 0
        for tname, p, f in items:
            where[tname] = (bname, p, off, f)
            off += f
        shapes[bname] = (pb, off)
    return where, shapes


_WHERE, _BLOB_SHAPES = _blob_layout()


# ---------------------------------------------------------------- host side

def _percore_inputs(xb, W1, b1, W2, b2, D1, db1, D2, db2, D3, db3):
    """Small per-core operands, precomputed in float64, packed into blobs."""
    X = xb.astype(np.float64)                      # [256, 16]
    n = float(N)
    sigma = X.sum(0)
    wt = W1[D:, :, 0] + W1[D:, :, 1]               # [16,32]
    alpha = W1[:D, :, 0] + W1[:D, :, 1] + W1[:D, :, 2]
    beta = W1[D:, :, 2]
    abias = alpha.T @ X.T + beta.T @ (X.T ** 2)    # [32,256]
    gamma = W1[:D, :, 3] / n + W1[D:, :, 3] * sigma[:, None] / n
    k = (W1[:D, :, 4].T @ (sigma / n**2)
         + W1[D:, :, 4].T @ (sigma**2 / n**2) + b1)
    cp = gamma.T @ X.T + k[:, None]                # [32,256]
    XT = X.T

    WtBD = np.zeros((A * D, 128))
    for a in range(A):
        WtBD[a * D:(a + 1) * D, a * NH:(a + 1) * NH] = wt
    Xr = X.reshape(G, A, D).transpose(1, 2, 0).reshape(A * D, G)
    Cpp = cp.reshape(NH, G, A).transpose(2, 0, 1).reshape(128, G)

    def blockdiag(M):
        out = np.zeros((128, 128))
        for a in range(A):
            out[a * NH:(a + 1) * NH, a * NH:(a + 1) * NH] = M
        return out

    vals = {
        'XT4': np.tile(XT, (A, 1)),
        'cpm2': np.tile(cp, (1, 2)),
        'WtBD': WtBD, 'Xr': Xr,
        'I32r4': np.tile(np.eye(NH), (1, A)),
        'Cpp': Cpp,
        'WB0': blockdiag(W2[:, :, 0]), 'WB1': blockdiag(W2[:, :, 1]),
        'WB3': blockdiag(W2[:, :, 3] / n),
        'P32': np.tile(np.eye(NH), (A, 1)),
        'PW': np.tile(np.eye(NH), (A, 1)) @ (W2[:, :, 4] / n**2),
        'W01': W2[:, :, 0] + W2[:, :, 1], 'W22': W2[:, :, 2],
        'W24': W2[:, :, 4] / n**2,
        'wt16': wt,
        'X2Tr': _reorder_ag(XT ** 2),
        'cpr': _reorder_ag(cp),
        'abiasr': _reorder_ag(abias),
        'b2c': b2[:, None],
        'D1m': D1, 'db1m': db1[:, None],
        'D2m': D2, 'db2m': db2[:, None],
        'D3m': D3, 'db3m': db3[:, None],
    }
    blobs = {bn: np.zeros(sh, dtype=F32) for bn, sh in _BLOB_SHAPES.items()}
    for tname, (bn, p, off, f) in _WHERE.items():
        v = np.asarray(vals[tname], dtype=np.float64)
        assert v.shape == (p, f), (tname, v.shape, (p, f))
        blobs[bn][0:p, off:off + f] = v.astype(F32)
    return blobs


# -------------------------------------------------------------- device side

def build_program():
    if 'nc' in _PROG_CACHE:
        return _PROG_CACHE['nc']

    from contextlib import ExitStack
    import concourse.bacc as bacc
    import concourse.tile as tile
    from concourse import mybir

    f32 = mybir.dt.float32
    f32r = mybir.dt.float32r
    bf16 = mybir.dt.bfloat16
    AF = mybir.ActivationFunctionType
    ALU = mybir.AluOpType

    nc = bacc.Bacc(trn_type="TRN2", target_bir_lowering=False)
    dram = {bn: nc.dram_tensor(bn, list(sh), f32, kind="ExternalInput")
            for bn, sh in _BLOB_SHAPES.items()}
    yout_d = nc.dram_tensor("yout", [1, 1], f32, kind="ExternalOutput")

    with tile.TileContext(nc) as tc:
        ctx = ExitStack()
        consts = ctx.enter_context(tc.tile_pool(name="consts", bufs=1))
        bt = {}
        for bn, sh in _BLOB_SHAPES.items():
            t = consts.tile(list(sh), f32, name=f"sb_{bn}")
            nc.default_dma_engine.dma_start(out=t, in_=dram[bn].ap())
            bt[bn] = t
        sb = {tn: bt[bn][0:p, off:off + f]
              for tn, (bn, p, off, f) in _WHERE.items()}

        big = ctx.enter_context(tc.tile_pool(name="big", bufs=1))
        zero256 = big.tile([128, 256], f32, name="zero256")
        nc.vector.memset(zero256, 0.0)
        H4 = big.tile([128, G * N], f32r, name="H4")
        HT4 = big.tile([128, G * N], f32r, name="HT4")
        r4 = big.tile([128, G], f32, name="r4")
        acc = big.tile([128, G], f32, name="acc")

        lhsT_pool = ctx.enter_context(tc.tile_pool(name="lhsT", bufs=4))
        psA_pool = ctx.enter_context(
            tc.tile_pool(name="psA", bufs=4, space="PSUM"))
        psU_pool = ctx.enter_context(
            tc.tile_pool(name="psU", bufs=3, space="PSUM"))
        scrap_pool = ctx.enter_context(tc.tile_pool(name="scrap", bufs=3))
        small = ctx.enter_context(tc.tile_pool(name="small", bufs=1))
        psm_pool = ctx.enter_context(
            tc.tile_pool(name="psm", bufs=1, space="PSUM"))

        def psm():
            return psm_pool.tile([128, 512], f32, name="psm")

        # fp32r-consumed operands need a rounding producer
        xt4r = consts.tile([64, 256], f32r, name="xt4r")
        nc.gpsimd.tensor_copy(xt4r, sb['XT4'])
        cpm2r = consts.tile([32, 512], f32r, name="cpm2r")
        nc.gpsimd.tensor_copy(cpm2r, sb['cpm2'])
        i32r = consts.tile([32, 128], f32r, name="i32r")
        nc.gpsimd.tensor_copy(i32r, sb['I32r4'])
        wb0r = consts.tile([128, 128], f32r, name="wb0r")
        nc.gpsimd.tensor_copy(wb0r, sb['WB0'])
        wb1r = consts.tile([128, 128], f32r, name="wb1r")
        nc.gpsimd.tensor_copy(wb1r, sb['WB1'])

        # ---- Small-phase prefix (independent of H; overlaps phase A).
        # All [32, 256] tensors here use (a, g) column order: column a*G+g
        # holds logical row i = 4g+a, so the [32,256] -> [128,64] repack is
        # 4 contiguous DMAs.
        psS = psm()[0:32, 0:256]
        nc.tensor.matmul(psS, lhsT=sb['wt16'], rhs=sb['X2Tr'],
                         start=True, stop=True, skip_group_check=True)
        t0 = small.tile([32, 256], f32, name="t0")
        nc.vector.tensor_add(t0, psS, sb['cpr'])
        dn = small.tile([32, 256], f32, name="dn")
        nc.gpsimd.tensor_scalar_max(dn, t0, 0.0)
        t1 = small.tile([32, 256], f32, name="t1")
        nc.vector.tensor_add(t1, t0, sb['abiasr'])
        dg = small.tile([32, 256], f32, name="dg")
        nc.gpsimd.tensor_scalar_max(dg, t1, 0.0)
        hdc = small.tile([32, 256], f32, name="hdc")
        nc.vector.tensor_sub(hdc, dg, dn)
        hdc4 = small.tile([128, G], f32, name="hdc4")
        for a in range(A):
            nc.default_dma_engine.dma_start(
                out=hdc4[a * NH:(a + 1) * NH, :],
                in_=hdc[:, a * G:(a + 1) * G])
        psQ = psm()[0:32, 0:256]
        nc.tensor.matmul(psQ, lhsT=sb['W01'], rhs=hdc,
                         start=True, stop=False, skip_group_check=True)
        nc.tensor.matmul(psQ, lhsT=sb['W22'], rhs=dg,
                         start=False, stop=True, skip_group_check=True)
        qsb = small.tile([32, 256], f32, name="qsb")
        nc.scalar.copy(qsb, psQ)
        psU2 = psm()[0:32, 0:256]
        nc.tensor.matmul(psU2, lhsT=sb['W01'], rhs=dn,
                         start=True, stop=True, skip_group_check=True)
        u2sb = small.tile([32, 256], f32, name="u2sb")
        nc.scalar.copy(u2sb, psU2)

        # ---- Phase A: H and H^T tiles; 2 row-groups share one PSUM bank
        for c in range(G // 2):
            g0, g1 = 2 * c, 2 * c + 1
            ps = psA_pool.tile([128, 512], f32, name="psA")
            for k, g in ((0, g0), (1, g1)):
                lhsTg = lhsT_pool.tile([64, 128], f32r, name="lhsTg")
                nc.gpsimd.tensor_scalar_mul(lhsTg, sb['WtBD'],
                                            sb['Xr'][:, g:g + 1])
                half = ps[:, k * N:(k + 1) * N]
                # start=True zeroes the whole 2KB PSUM zero-region (bank),
                # so only the first matmul in this bank may set it; the
                # second half is zeroed by its own first write (pending).
                nc.tensor.matmul(half, lhsT=lhsTg, rhs=xt4r,
                                 start=(k == 0), stop=False,
                                 skip_group_check=True)
                # H = relu(S + c'_i) (row bias per partition); row sums
                # accumulate into r4 for the later rho/kappa biases.
                if g % 16 == 15:
                    nc.scalar.activation(
                        out=H4[:, g * N:(g + 1) * N], in_=half, func=AF.Relu,
                        bias=sb['Cpp'][:, g:g + 1], accum_out=r4[:, g:g + 1])
                else:
                    nc.vector.scalar_tensor_tensor(
                        H4[:, g * N:(g + 1) * N], half, sb['Cpp'][:, g:g + 1],
                        zero256, ALU.add, ALU.max, accum_out=r4[:, g:g + 1])
            # S + c'_j for both halves in one K=32 matmul, one wide relu
            nc.tensor.matmul(ps, lhsT=i32r, rhs=cpm2r,
                             start=False, stop=True, skip_group_check=True)
            nc.scalar.activation(out=HT4[:, g0 * N:(g1 + 1) * N], in_=ps,
                                 func=AF.Relu)

        # ---- Small-phase suffix: rho/kappa biases (needs all of r4)
        r4hat = small.tile([128, G], f32, name="r4hat")
        nc.vector.tensor_add(r4hat, r4, hdc4)
        rsum = small.tile([128, 1], f32, name="rsum")
        nc.vector.tensor_reduce(out=rsum, in_=r4hat,
                                axis=mybir.AxisListType.X, op=ALU.add)
        psT = psm()
        nc.tensor.matmul(psT[0:32, 4:5], lhsT=sb['PW'], rhs=rsum,
                         start=True, stop=True, skip_group_check=True)
        ksb = small.tile([32, 1], f32, name="ksb")
        nc.scalar.activation(out=ksb, in_=psT[0:32, 4:5], func=AF.Identity,
                             bias=sb['b2c'])
        nc.tensor.matmul(psT[:, 8:9], lhsT=sb['I32r4'], rhs=ksb,
                         start=True, stop=True, skip_group_check=True)
        krep = small.tile([128, 1], f32, name="krep")
        nc.scalar.copy(krep, psT[:, 8:9])
        nc.tensor.matmul(psT[:, 64:64 + G], lhsT=sb['WB3'], rhs=r4hat,
                         start=True, stop=True, skip_group_check=True)
        rhoka = small.tile([128, G], f32, name="rhoka")
        nc.scalar.activation(out=rhoka, in_=psT[:, 64:64 + G],
                             func=AF.Identity, bias=krep)

        # corr path ((a,g) order throughout) — runs parallel with phase B
        rhokr = small.tile([32, 256], f32, name="rhokr")
        for a in range(A):
            nc.default_dma_engine.dma_start(
                out=rhokr[:, a * G:(a + 1) * G],
                in_=rhoka[a * NH:(a + 1) * NH, :])
        uii = small.tile([32, 256], f32, name="uii")
        nc.gpsimd.tensor_add(uii, u2sb, rhokr)
        t3 = small.tile([32, 256], f32, name="t3")
        nc.gpsimd.tensor_add(t3, uii, qsb)
        scrapS = small.tile([32, 256], f32, name="scrapS")
        cA = small.tile([32, 1], f32, name="cA")
        nc.vector.tensor_scalar(scrapS, t3, 0.0, None, ALU.max, ALU.add,
                                accum_out=cA)
        scrapS2 = small.tile([32, 256], f32, name="scrapS2")
        cB = small.tile([32, 1], f32, name="cB")
        nc.vector.tensor_scalar(scrapS2, uii, 0.0, None, ALU.max, ALU.add,
                                accum_out=cB)
        corr = small.tile([32, 1], f32, name="corr")
        nc.vector.tensor_sub(corr, cA, cB)

        # ---- Phase B: channel mix + fused bias-relu-rowsum.
        # DVE's fused op is cheaper (392ns vs 585ns exclusive), so it takes
        # the larger share.
        for g in range(G):
            ps = psU_pool.tile([128, N], f32, name="psU")
            sl = slice(g * N, (g + 1) * N)
            nc.tensor.matmul(ps, lhsT=wb0r, rhs=H4[:, sl],
                             start=True, stop=False, skip_group_check=True)
            nc.tensor.matmul(ps, lhsT=wb1r, rhs=HT4[:, sl],
                             start=False, stop=True, skip_group_check=True)
            scrap = scrap_pool.tile([128, N], f32, name="scrap")
            if g % 2 == 0:
                nc.vector.scalar_tensor_tensor(
                    scrap, ps, rhoka[:, g:g + 1], zero256,
                    ALU.add, ALU.max, accum_out=acc[:, g:g + 1])
            else:
                nc.scalar.activation(out=scrap, in_=ps, func=AF.Relu,
                                     bias=rhoka[:, g:g + 1],
                                     accum_out=acc[:, g:g + 1])

        # ---- Pooling + MLP head
        accred = small.tile([128, 1], f32, name="accred")
        nc.vector.tensor_reduce(out=accred, in_=acc,
                                axis=mybir.AxisListType.X, op=ALU.add)
        psY = psm()
        nc.tensor.matmul(psY[0:32, 0:1], lhsT=sb['P32'], rhs=accred,
                         start=True, stop=True, skip_group_check=True)
        p_sb = small.tile([32, 1], f32, name="p_sb")
        nc.scalar.activation(out=p_sb, in_=psY[0:32, 0:1], func=AF.Relu,
                             bias=corr)
        nc.tensor.matmul(psY[:, 4:5], lhsT=sb['D1m'], rhs=p_sb,
                         start=True, stop=True, skip_group_check=True)
        y1 = small.tile([128, 1], f32, name="y1")
        nc.scalar.activation(out=y1, in_=psY[:, 4:5], func=AF.Relu,
                             bias=sb['db1m'])
        nc.tensor.matmul(psY[:, 8:9], lhsT=sb['D2m'], rhs=y1,
                         start=True, stop=True, skip_group_check=True)
        y2 = small.tile([128, 1], f32, name="y2")
        nc.scalar.activation(out=y2, in_=psY[:, 8:9], func=AF.Relu,
                             bias=sb['db2m'])
        nc.tensor.matmul(psY[0:1, 12:13], lhsT=sb['D3m'], rhs=y2,
                         start=True, stop=True, skip_group_check=True)
        yo = small.tile([1, 1], f32, name="yo")
        nc.scalar.activation(out=yo, in_=psY[0:1, 12:13], func=AF.Identity,
                             bias=sb['db3m'])
        nc.default_dma_engine.dma_start(out=yout_d.ap(), in_=yo)

        ctx.close()

    nc.compile()
    _PROG_CACHE['nc'] = nc
    return nc


def make_in_maps(inputs):
    x = np.asarray(inputs['x'], dtype=F32)
    args = [np.asarray(inputs[k], dtype=np.float64) for k in
            ('W1', 'b1', 'W2', 'b2', 'D1', 'db1', 'D2', 'db2', 'D3', 'db3')]
    return [_percore_inputs(x[b], *args) for b in range(B)]


def kernel(**inputs) -> np.ndarray:
    from concourse.bass_utils import run_bass_kernel_spmd
    nc = build_program()
    in_maps = make_in_maps(inputs)
    res = run_bass_kernel_spmd(nc, in_maps, core_ids=list(range(B))).results
    return np.concatenate([res[b]['yout'].reshape(1, 1) for b in range(B)],
                          axis=0).astype(F32)


# revision 33
# speedup vs baseline: 6846.8522x; 1.1126x over previous
"""Trainium2 Bass kernel for MiniEq2Net (gnn_message_passing).

Math (validated against the jax reference in float64, rel err ~3e-7):

Per batch b (X = x[b], [n=256, d=16]) the first eq-layer's input channels are
diag(X[:,d]) and X[:,d] outer X[:,d], so layer 1 collapses to
    G1[s] = S(s) + c'_{s,i} (row-broadcast) + delta_ij a_{s,i}
with S(s) = X diag(wt_s) X^T (symmetric, one K=64 matmul per 4-row group in a
packed (a=i%4, s) x (j) layout), and the diagonal handled exactly via tiny
[32,256] side computations (dn/dg/Hdc).  Layer 2 + pooling becomes two K=128
block-diagonal channel-mix matmuls over relu'd H and H^T plus a fused
relu-accumulate, with the diagonal / rowsum / total-sum basis terms folded
into per-partition biases and a closed-form correction.

Sharding: pure data parallel, one batch element per NeuronCore (B=8, 8 cores).
All heavy intermediates (H, H^T: 16MB) stay in SBUF; HBM traffic is ~0.6MB of
host-precomputed small operands per core, packed into 3 blob tensors so the
input load is 3 large DMAs instead of ~22 small ones.
"""

import numpy as np

N = 256          # n (graph nodes)
D = 16           # input channel count
NH = 32          # hidden channels
A = 4            # row-packing factor: partition p = a*32+s, row i = 4*g+a
G = N // A       # 64 row-groups
B = 8            # batch == cores
F32 = np.float32

_PROG_CACHE = {}


def _reorder_ag(arr):
    """Permute the trailing i axis (len 256) into (a, g) order:
    out[..., a*G+g] = arr[..., 4*g+a]."""
    sh = arr.shape[:-1]
    return arr.reshape(*sh, G, A).swapaxes(-1, -2).reshape(*sh, N)


# Blob packing: blob_name -> (partition_count, [(tensor_name, P, F), ...])
_BLOBS = {
    'blob128': (128, [
        ('Cpp', 128, G), ('WB0', 128, 128), ('WB1', 128, 128),
        ('WB3', 128, 128), ('P32', 128, 32), ('D2m', 128, 128),
        ('db1m', 128, 1), ('db2m', 128, 1), ('D3m', 128, 1),
        ('PW', 128, 32),
    ]),
    'blob64': (64, [
        ('XT4', 64, 256), ('WtBD', 64, 128), ('Xr', 64, G),
    ]),
    'blob32': (32, [
        ('cpm2', 32, 512), ('I32r4', 32, 128),
        ('W01', 32, 32), ('W22', 32, 32), ('W24', 32, 32),
        ('wt16', 16, 32), ('X2Tr', 16, 256), ('cpr', 32, 256),
        ('abiasr', 32, 256), ('b2c', 32, 1), ('D1m', 32, 128),
        ('db3m', 1, 1),
    ]),
}


def _blob_layout():
    where, shapes = {}, {}
    for bname, (pb, items) in _BLOBS.items():
        off = 0
        for tname, p, f in items:
            where[tname] = (bname, p, off, f)
            off += f
        shapes[bname] = (pb, off)
    return where, shapes


_WHERE, _BLOB_SHAPES = _blob_layout()


# ---------------------------------------------------------------- host side

def _percore_inputs(xb, W1, b1, W2, b2, D1, db1, D2, db2, D3, db3):
    """Small per-core operands, precomputed in float64, packed into blobs."""
    X = xb.astype(np.float64)                      # [256, 16]
    n = float(N)
    sigma = X.sum(0)
    wt = W1[D:, :, 0] + W1[D:, :, 1]               # [16,32]
    alpha = W1[:D, :, 0] + W1[:D, :, 1] + W1[:D, :, 2]
    beta = W1[D:, :, 2]
    abias = alpha.T @ X.T + beta.T @ (X.T ** 2)    # [32,256]
    gamma = W1[:D, :, 3] / n + W1[D:, :, 3] * sigma[:, None] / n
    k = (W1[:D, :, 4].T @ (sigma / n**2)
         + W1[D:, :, 4].T @ (sigma**2 / n**2) + b1)
    cp = gamma.T @ X.T + k[:, None]                # [32,256]
    XT = X.T

    WtBD = np.zeros((A * D, 128))
    for a in range(A):
        WtBD[a * D:(a + 1) * D, a * NH:(a + 1) * NH] = wt
    Xr = X.reshape(G, A, D).transpose(1, 2, 0).reshape(A * D, G)
    Cpp = cp.reshape(NH, G, A).transpose(2, 0, 1).reshape(128, G)

    def blockdiag(M):
        out = np.zeros((128, 128))
        for a in range(A):
            out[a * NH:(a + 1) * NH, a * NH:(a + 1) * NH] = M
        return out

    vals = {
        'XT4': np.tile(XT, (A, 1)),
        'cpm2': np.tile(cp, (1, 2)),
        'WtBD': WtBD, 'Xr': Xr,
        'I32r4': np.tile(np.eye(NH), (1, A)),
        'Cpp': Cpp,
        'WB0': blockdiag(W2[:, :, 0]), 'WB1': blockdiag(W2[:, :, 1]),
        'WB3': blockdiag(W2[:, :, 3] / n),
        'P32': np.tile(np.eye(NH), (A, 1)),
        'PW': np.tile(np.eye(NH), (A, 1)) @ (W2[:, :, 4] / n**2),
        'W01': W2[:, :, 0] + W2[:, :, 1], 'W22': W2[:, :, 2],
        'W24': W2[:, :, 4] / n**2,
        'wt16': wt,
        'X2Tr': _reorder_ag(XT ** 2),
        'cpr': _reorder_ag(cp),
        'abiasr': _reorder_ag(abias),
        'b2c': b2[:, None],
        'D1m': D1, 'db1m': db1[:, None],
        'D2m': D2, 'db2m': db2[:, None],
        'D3m': D3, 'db3m': db3[:, None],
    }
    blobs = {bn: np.zeros(sh, dtype=F32) for bn, sh in _BLOB_SHAPES.items()}
    for tname, (bn, p, off, f) in _WHERE.items():
        v = np.asarray(vals[tname], dtype=np.float64)
        assert v.shape == (p, f), (tname, v.shape, (p, f))
        blobs[bn][0:p, off:off + f] = v.astype(F32)
    return blobs


# -------------------------------------------------------------- device side

def build_program():
    if 'nc' in _PROG_CACHE:
        return _PROG_CACHE['nc']

    from contextlib import ExitStack
    import concourse.bacc as bacc
    import concourse.tile as tile
    from concourse import mybir

    f32 = mybir.dt.float32
    f32r = mybir.dt.float32r
    bf16 = mybir.dt.bfloat16
    AF = mybir.ActivationFunctionType
    ALU = mybir.AluOpType

    nc = bacc.Bacc(trn_type="TRN2", target_bir_lowering=False)
    dram = {bn: nc.dram_tensor(bn, list(sh), f32, kind="ExternalInput")
            for bn, sh in _BLOB_SHAPES.items()}
    yout_d = nc.dram_tensor("yout", [1, 1], f32, kind="ExternalOutput")

    with tile.TileContext(nc) as tc:
        ctx = ExitStack()
        consts = ctx.enter_context(tc.tile_pool(name="consts", bufs=1))
        bt = {}
        for bn, sh in _BLOB_SHAPES.items():
            t = consts.tile(list(sh), f32, name=f"sb_{bn}")
            nc.default_dma_engine.dma_start(out=t, in_=dram[bn].ap())
            bt[bn] = t
        sb = {tn: bt[bn][0:p, off:off + f]
              for tn, (bn, p, off, f) in _WHERE.items()}

        big = ctx.enter_context(tc.tile_pool(name="big", bufs=1))
        zero256 = big.tile([128, 256], f32, name="zero256")
        nc.vector.memset(zero256, 0.0)
        H4 = big.tile([128, G * N], f32r, name="H4")
        HT4 = big.tile([128, G * N], f32r, name="HT4")
        r4 = big.tile([128, G], f32, name="r4")
        acc = big.tile([128, G], f32, name="acc")

        lhsT_pool = ctx.enter_context(tc.tile_pool(name="lhsT", bufs=8))
        scrap_pool = ctx.enter_context(tc.tile_pool(name="scrap", bufs=6))
        small = ctx.enter_context(tc.tile_pool(name="small", bufs=1))
        psm_pool = ctx.enter_context(
            tc.tile_pool(name="psm", bufs=1, space="PSUM"))

        def psm():
            return psm_pool.tile([128, 512], f32, name="psm")

        # fp32r-consumed operands need a rounding producer
        xt4r = consts.tile([64, 256], f32r, name="xt4r")
        nc.gpsimd.tensor_copy(xt4r, sb['XT4'])
        cpm2r = consts.tile([32, 512], f32r, name="cpm2r")
        nc.gpsimd.tensor_copy(cpm2r, sb['cpm2'])
        i32r = consts.tile([32, 128], f32r, name="i32r")
        nc.gpsimd.tensor_copy(i32r, sb['I32r4'])

        # ---- Small-phase prefix (independent of H; overlaps phase A).
        # All [32, 256] tensors here use (a, g) column order: column a*G+g
        # holds logical row i = 4g+a, so the [32,256] -> [128,64] repack is
        # 4 contiguous DMAs.
        psS = psm()[0:32, 0:256]
        nc.tensor.matmul(psS, lhsT=sb['wt16'], rhs=sb['X2Tr'],
                         start=True, stop=True, skip_group_check=True)
        t0 = small.tile([32, 256], f32, name="t0")
        nc.vector.tensor_add(t0, psS, sb['cpr'])
        dn = small.tile([32, 256], f32, name="dn")
        nc.gpsimd.tensor_scalar_max(dn, t0, 0.0)
        t1 = small.tile([32, 256], f32, name="t1")
        nc.vector.tensor_add(t1, t0, sb['abiasr'])
        dg = small.tile([32, 256], f32, name="dg")
        nc.gpsimd.tensor_scalar_max(dg, t1, 0.0)
        hdc = small.tile([32, 256], f32, name="hdc")
        nc.vector.tensor_sub(hdc, dg, dn)
        hdc4 = small.tile([128, G], f32, name="hdc4")
        for a in range(A):
            nc.default_dma_engine.dma_start(
                out=hdc4[a * NH:(a + 1) * NH, :],
                in_=hdc[:, a * G:(a + 1) * G])
        psQ = psm()[0:32, 0:256]
        nc.tensor.matmul(psQ, lhsT=sb['W01'], rhs=hdc,
                         start=True, stop=False, skip_group_check=True)
        nc.tensor.matmul(psQ, lhsT=sb['W22'], rhs=dg,
                         start=False, stop=True, skip_group_check=True)
        qsb = small.tile([32, 256], f32, name="qsb")
        nc.scalar.copy(qsb, psQ)
        psU2 = psm()[0:32, 0:256]
        nc.tensor.matmul(psU2, lhsT=sb['W01'], rhs=dn,
                         start=True, stop=True, skip_group_check=True)
        u2sb = small.tile([32, 256], f32, name="u2sb")
        nc.scalar.copy(u2sb, psU2)

        # ---- Phase A: H and H^T tiles; 2 row-groups share one PSUM bank
        psA_ctx = ExitStack()
        psA_pool = psA_ctx.enter_context(
            tc.tile_pool(name="psA", bufs=7, space="PSUM"))
        for c in range(G // 2):
            g0, g1 = 2 * c, 2 * c + 1
            ps = psA_pool.tile([128, 512], f32, name="psA")
            for k, g in ((0, g0), (1, g1)):
                lhsTg = lhsT_pool.tile([64, 128], f32r, name="lhsTg")
                nc.gpsimd.tensor_scalar_mul(lhsTg, sb['WtBD'],
                                            sb['Xr'][:, g:g + 1])
                half = ps[:, k * N:(k + 1) * N]
                # start=True zeroes the whole 2KB PSUM zero-region (bank),
                # so only the first matmul in this bank may set it; the
                # second half is zeroed by its own first write (pending).
                nc.tensor.matmul(half, lhsT=lhsTg, rhs=xt4r,
                                 start=(k == 0), stop=False,
                                 skip_group_check=True)
                # H = relu(S + c'_i) (row bias per partition); row sums
                # accumulate into r4 for the later rho/kappa biases.
                if g % 16 == 15:
                    nc.scalar.activation(
                        out=H4[:, g * N:(g + 1) * N], in_=half, func=AF.Relu,
                        bias=sb['Cpp'][:, g:g + 1], accum_out=r4[:, g:g + 1])
                else:
                    nc.vector.scalar_tensor_tensor(
                        H4[:, g * N:(g + 1) * N], half, sb['Cpp'][:, g:g + 1],
                        zero256, ALU.add, ALU.max, accum_out=r4[:, g:g + 1])
            # S + c'_j for both halves in one K=32 matmul, one wide relu
            nc.tensor.matmul(ps, lhsT=i32r, rhs=cpm2r,
                             start=False, stop=True, skip_group_check=True)
            nc.scalar.activation(out=HT4[:, g0 * N:(g1 + 1) * N], in_=ps,
                                 func=AF.Relu)

        psA_ctx.close()
        psU_pool = ctx.enter_context(
            tc.tile_pool(name="psU", bufs=7, space="PSUM"))

        # phase-B-only fp32r operands: convert after phase A has started
        wb0r = consts.tile([128, 128], f32r, name="wb0r")
        nc.gpsimd.tensor_copy(wb0r, sb['WB0'])
        wb1r = consts.tile([128, 128], f32r, name="wb1r")
        nc.gpsimd.tensor_copy(wb1r, sb['WB1'])

        # ---- Small-phase suffix: rho/kappa biases (needs all of r4)
        r4hat = small.tile([128, G], f32, name="r4hat")
        nc.vector.tensor_add(r4hat, r4, hdc4)
        rsum = small.tile([128, 1], f32, name="rsum")
        nc.vector.tensor_reduce(out=rsum, in_=r4hat,
                                axis=mybir.AxisListType.X, op=ALU.add)
        psT = psm()
        nc.tensor.matmul(psT[0:32, 4:5], lhsT=sb['PW'], rhs=rsum,
                         start=True, stop=True, skip_group_check=True)
        ksb = small.tile([32, 1], f32, name="ksb")
        nc.scalar.activation(out=ksb, in_=psT[0:32, 4:5], func=AF.Identity,
                             bias=sb['b2c'])
        nc.tensor.matmul(psT[:, 8:9], lhsT=sb['I32r4'], rhs=ksb,
                         start=True, stop=True, skip_group_check=True)
        krep = small.tile([128, 1], f32, name="krep")
        nc.scalar.copy(krep, psT[:, 8:9])
        nc.tensor.matmul(psT[:, 64:64 + G], lhsT=sb['WB3'], rhs=r4hat,
                         start=True, stop=True, skip_group_check=True)
        rhoka = small.tile([128, G], f32, name="rhoka")
        nc.scalar.activation(out=rhoka, in_=psT[:, 64:64 + G],
                             func=AF.Identity, bias=krep)

        # corr path ((a,g) order throughout) — runs parallel with phase B
        rhokr = small.tile([32, 256], f32, name="rhokr")
        for a in range(A):
            nc.default_dma_engine.dma_start(
                out=rhokr[:, a * G:(a + 1) * G],
                in_=rhoka[a * NH:(a + 1) * NH, :])
        uii = small.tile([32, 256], f32, name="uii")
        nc.gpsimd.tensor_add(uii, u2sb, rhokr)
        t3 = small.tile([32, 256], f32, name="t3")
        nc.gpsimd.tensor_add(t3, uii, qsb)
        scrapS = small.tile([32, 256], f32, name="scrapS")
        cA = small.tile([32, 1], f32, name="cA")
        nc.vector.tensor_scalar(scrapS, t3, 0.0, None, ALU.max, ALU.add,
                                accum_out=cA)
        scrapS2 = small.tile([32, 256], f32, name="scrapS2")
        cB = small.tile([32, 1], f32, name="cB")
        nc.vector.tensor_scalar(scrapS2, uii, 0.0, None, ALU.max, ALU.add,
                                accum_out=cB)
        corr = small.tile([32, 1], f32, name="corr")
        nc.vector.tensor_sub(corr, cA, cB)

        # ---- Phase B: channel mix + fused bias-relu-rowsum.
        # DVE's fused op is cheaper (392ns vs 585ns exclusive), so it takes
        # the larger share.
        for g in range(G):
            ps = psU_pool.tile([128, N], f32, name="psU")
            sl = slice(g * N, (g + 1) * N)
            nc.tensor.matmul(ps, lhsT=wb0r, rhs=H4[:, sl],
                             start=True, stop=False, skip_group_check=True)
            nc.tensor.matmul(ps, lhsT=wb1r, rhs=HT4[:, sl],
                             start=False, stop=True, skip_group_check=True)
            scrap = scrap_pool.tile([128, N], f32, name="scrap")
            if g % 16 in (0, 2, 4, 6, 8, 9, 11, 13, 15):
                nc.vector.scalar_tensor_tensor(
                    scrap, ps, rhoka[:, g:g + 1], zero256,
                    ALU.add, ALU.max, accum_out=acc[:, g:g + 1])
            else:
                nc.scalar.activation(out=scrap, in_=ps, func=AF.Relu,
                                     bias=rhoka[:, g:g + 1],
                                     accum_out=acc[:, g:g + 1])

        # ---- Pooling + MLP head
        accred = small.tile([128, 1], f32, name="accred")
        nc.vector.tensor_reduce(out=accred, in_=acc,
                                axis=mybir.AxisListType.X, op=ALU.add)
        psY = psm()
        nc.tensor.matmul(psY[0:32, 0:1], lhsT=sb['P32'], rhs=accred,
                         start=True, stop=True, skip_group_check=True)
        p_sb = small.tile([32, 1], f32, name="p_sb")
        nc.scalar.activation(out=p_sb, in_=psY[0:32, 0:1], func=AF.Relu,
                             bias=corr)
        nc.tensor.matmul(psY[:, 4:5], lhsT=sb['D1m'], rhs=p_sb,
                         start=True, stop=True, skip_group_check=True)
        y1 = small.tile([128, 1], f32, name="y1")
        nc.scalar.activation(out=y1, in_=psY[:, 4:5], func=AF.Relu,
                             bias=sb['db1m'])
        nc.tensor.matmul(psY[:, 8:9], lhsT=sb['D2m'], rhs=y1,
                         start=True, stop=True, skip_group_check=True)
        y2 = small.tile([128, 1], f32, name="y2")
        nc.scalar.activation(out=y2, in_=psY[:, 8:9], func=AF.Relu,
                             bias=sb['db2m'])
        nc.tensor.matmul(psY[0:1, 12:13], lhsT=sb['D3m'], rhs=y2,
                         start=True, stop=True, skip_group_check=True)
        yo = small.tile([1, 1], f32, name="yo")
        nc.scalar.activation(out=yo, in_=psY[0:1, 12:13], func=AF.Identity,
                             bias=sb['db3m'])
        nc.default_dma_engine.dma_start(out=yout_d.ap(), in_=yo)

        ctx.close()

    nc.compile()
    _PROG_CACHE['nc'] = nc
    return nc


def make_in_maps(inputs):
    x = np.asarray(inputs['x'], dtype=F32)
    args = [np.asarray(inputs[k], dtype=np.float64) for k in
            ('W1', 'b1', 'W2', 'b2', 'D1', 'db1', 'D2', 'db2', 'D3', 'db3')]
    return [_percore_inputs(x[b], *args) for b in range(B)]


def kernel(**inputs) -> np.ndarray:
    from concourse.bass_utils import run_bass_kernel_spmd
    nc = build_program()
    in_maps = make_in_maps(inputs)
    res = run_bass_kernel_spmd(nc, in_maps, core_ids=list(range(B))).results
    return np.concatenate([res[b]['yout'].reshape(1, 1) for b in range(B)],
                          axis=0).astype(F32)
